# revision 27
# baseline (speedup 1.0000x reference)
"""Trainium2 Bass kernel for nn_Attention_27943057228498 (sparse token-pruning
attention, ViT-style EViT).

Strategy: pure data parallelism over batch — 32 batches over 8 NeuronCores,
4 per core, no collectives.

Numerics: the top-k token selection compares attention diagonal values whose
boundary gaps are as small as ~6e-6 relative, so everything feeding the
ranking (QK projection, scores, softmax row sums, diagonal) runs in exact
fp32 on the PE (4 cycles/row). The output path (V projection, attn@V,
output projection) only needs ~1e-3, so it runs in fp16 (1 cycle/row).

PE packing: heads are processed in pairs. The pair's K^T/Q^T slices live at
partitions 0:64 / 64:128 of the same tile, so their K=64 score matmuls run
concurrently in disjoint PE row groups; the pair's attn@V matmuls (M=64)
run concurrently in disjoint column groups into one PSUM tile.

Token selection without sorting: per batch, token score a_j gets rank
R_j = #{i: a_i > a_j} via compare + row-reduce; keep = R < num_kept (CLS is
forced kept by pinning a_0 = +1e30); output positions are an inclusive prefix
scan of the keep mask; rows are emitted with an indirect-DMA scatter whose
out-of-bounds indices (dropped tokens) are silently discarded. The whole
selection chain runs on DVE/GpSimd/DMA so the PE never waits on it.
"""

import numpy as np

import concourse.bass as bass
import concourse.bass_isa as bass_isa
import concourse.tile as tile
import concourse.mybir as mybir
from concourse import bacc
from concourse.bass_utils import run_bass_kernel_spmd

# ── problem constants ────────────────────────────────────────────────
B, N, C = 32, 577, 768
H = 12
HD = C // H              # 64
NCORES = 8
BL = B // NCORES         # 4 batches per core
SCALE = HD ** -0.5       # 0.125 (exact power of two)

P = 128
TOK_TILES = [(0, 128), (128, 128), (256, 128), (384, 128), (512, 65)]  # 577
CT = C // P              # 6 channel tiles
NPAD = 640               # 577 padded to 5*128 for the rank machinery
BIG = 1.0e9              # scatter index for dropped rows (exact in fp32)
NEG = -1.0e30            # pad value below any real score

F32 = mybir.dt.float32
F16 = mybir.dt.float16
U32 = mybir.dt.uint32
OP = mybir.AluOpType
ACTF = mybir.ActivationFunctionType


def _dedupe_ldweights(nc):
    """Remove back-to-back duplicate PE Ldweights (same weights AP + array
    tile) so repeated matmuls on one stationary operand pay one load.

    Only deletes an Ldweights when (a) it has no semaphore waits/updates of
    its own, and (b) the PE weight state for its array-tile region is
    provably identical (no intervening Ldweights / self-loading Matmult
    overlapping that region).
    """

    def region(inst):
        tp = inst.tile_position or (0, 0)
        ts = inst.tile_size or (128, 128)
        return (tp[0], tp[0] + ts[0], tp[1], tp[1] + ts[1])

    def overlaps(r1, r2):
        return r1[0] < r2[1] and r2[0] < r1[1] and r1[2] < r2[3] and r2[2] < r1[3]

    removed = 0
    for blk in nc.m.functions[0].blocks:
        state = []   # list of (region, signature)
        keep_list = []
        for inst in blk.instructions:
            if not isinstance(inst, (mybir.InstLdweights, mybir.InstMatmult)):
                keep_list.append(inst)
                continue
            if isinstance(inst, mybir.InstLdweights):
                sig = (str(inst.ins[0]), inst.tile_position, inst.tile_size,
                       inst.is_transpose)
                r = region(inst)
                if (not inst.has_wait() and not inst.has_update()
                        and any(overlaps(r, r2) and s2 == sig and r2 == r
                                for r2, s2 in state)):
                    removed += 1
                    continue     # drop duplicate load
                state = [(r2, s2) for r2, s2 in state if not overlaps(r, r2)]
                state.append((r, sig))
                keep_list.append(inst)
            else:
                # self-loading matmul (fp32) clobbers its region
                if getattr(inst, "ldweights", None) is not False:
                    r = region(inst)
                    state = [(r2, s2) for r2, s2 in state
                             if not overlaps(r, r2)]
                keep_list.append(inst)
        if removed:
            blk.instructions[:] = keep_list
    return removed


def _chunks(total, limit=512):
    out = []
    c0 = 0
    while c0 < total:
        cn = min(limit, total - c0)
        out.append((c0, cn))
        c0 += cn
    return out


def build(num_kept: int):
    nc = bacc.Bacc("TRN2", target_bir_lowering=False, debug=False,
                   num_devices=NCORES)

    xt_e = nc.dram_tensor("xt", [BL, C, N], F32, kind="ExternalInput")
    xt16_e = nc.dram_tensor("xt16", [BL, C, N], F16, kind="ExternalInput")
    xr_e = nc.dram_tensor("xr", [BL, N, C], F32, kind="ExternalInput")
    wq_e = nc.dram_tensor("wq", [C, C], F32, kind="ExternalInput")
    wk_e = nc.dram_tensor("wk", [C, C], F32, kind="ExternalInput")
    wv16_e = nc.dram_tensor("wv16", [C, C], F16, kind="ExternalInput")
    wp16_e = nc.dram_tensor("wp16", [C, C], F16, kind="ExternalInput")
    bias_e = nc.dram_tensor("biasb", [P, C], F32, kind="ExternalInput")
    hsel_e = nc.dram_tensor("hsel", [C, H], F32, kind="ExternalInput")
    out_e = nc.dram_tensor("out", [BL, num_kept, C], F32, kind="ExternalOutput")
    out_flat = out_e.ap().rearrange("b n c -> (b n) c")

    from contextlib import ExitStack
    with tile.TileContext(nc) as tc, ExitStack() as ctx:
        wpool = ctx.enter_context(tc.tile_pool(name="weights", bufs=1))
        xpool = ctx.enter_context(tc.tile_pool(name="x", bufs=1))
        qkpool = ctx.enter_context(tc.tile_pool(name="qk", bufs=1))
        vpool = ctx.enter_context(tc.tile_pool(name="v", bufs=1))
        opool = ctx.enter_context(tc.tile_pool(name="o", bufs=1))
        ppool = ctx.enter_context(tc.tile_pool(name="p", bufs=2))
        spool = ctx.enter_context(tc.tile_pool(name="small", bufs=1))
        ypool = ctx.enter_context(tc.tile_pool(name="y", bufs=2))
        dpool = ctx.enter_context(tc.tile_pool(name="dram", bufs=2,
                                               space="DRAM"))
        pspool = ctx.enter_context(tc.tile_pool(name="ps", bufs=2,
                                                space="PSUM"))
        psav = ctx.enter_context(tc.tile_pool(name="psav", bufs=2,
                                              space="PSUM"))

        # ── resident weights / constants ─────────────────────────────
        wq_t, wk_t, wv_t, wp_t, hsel_t = [], [], [], [], []
        for i in range(CT):
            w1 = wpool.tile([P, C], F32, tag=f"wq{i}")
            nc.sync.dma_start(w1[:], wq_e.ap()[i * P:(i + 1) * P, :])
            wq_t.append(w1)
            w2 = wpool.tile([P, C], F32, tag=f"wk{i}")
            nc.sync.dma_start(w2[:], wk_e.ap()[i * P:(i + 1) * P, :])
            wk_t.append(w2)
            w3 = wpool.tile([P, C], F16, tag=f"wv{i}")
            nc.sync.dma_start(w3[:], wv16_e.ap()[i * P:(i + 1) * P, :])
            wv_t.append(w3)
            w4 = wpool.tile([P, C], F16, tag=f"wp{i}")
            nc.sync.dma_start(w4[:], wp16_e.ap()[i * P:(i + 1) * P, :])
            wp_t.append(w4)
            w5 = wpool.tile([P, H], F32, tag=f"hs{i}")
            nc.sync.dma_start(w5[:], hsel_e.ap()[i * P:(i + 1) * P, :])
            hsel_t.append(w5)
        bias_t = wpool.tile([P, C], F32, tag="bias")
        nc.sync.dma_start(bias_t[:], bias_e.ap())
        zrow = wpool.tile([1, NPAD], F32, tag="zrow")
        nc.vector.memset(zrow[:], 0.0)
        ones128 = wpool.tile([P, 1], F32, tag="ones128")
        nc.vector.memset(ones128[:], 1.0)

        def load_x(b):
            xt_t, xt16_t = [], []
            for i in range(CT):
                t1 = xpool.tile([P, N], F32, tag=f"xt{i}", name=f"xt{i}")
                nc.sync.dma_start(t1[:], xt_e.ap()[b, i * P:(i + 1) * P, :])
                xt_t.append(t1)
                t2 = xpool.tile([P, N], F16, tag=f"xt16{i}", name=f"xt16{i}")
                nc.sync.dma_start(t2[:], xt16_e.ap()[b, i * P:(i + 1) * P, :])
                xt16_t.append(t2)
            return xt_t, xt16_t

        next_x = load_x(0)
        for b in range(BL):
            xt_t, xt16_t = next_x

            # ── Q/K projections (fp32): qT/kT[mo] = (C-tile, N) ──────
            qT, kT = [], []
            for w_t, dst in ((wq_t, qT), (wk_t, kT)):
                for mo in range(CT):
                    ps = pspool.tile([P, C], F32, tag="bigps")
                    for kc in range(CT):
                        for (c0, cn) in _chunks(N):
                            nc.tensor.matmul(
                                ps[:, c0:c0 + cn],
                                lhsT=w_t[kc][:, mo * P:(mo + 1) * P],
                                rhs=xt_t[kc][:, c0:c0 + cn],
                                start=(kc == 0), stop=(kc == CT - 1))
                    sb = qkpool.tile([P, N], F32,
                                     tag=f"{'q' if dst is qT else 'k'}T{mo}")
                    nc.scalar.copy(sb[:], ps[:, :N])
                    dst.append(sb)

            # ── V projection (fp16) → v16[mt] = (m, C) ───────────────
            v16 = []
            for mt, (t0, tn) in enumerate(TOK_TILES):
                ps = pspool.tile([P, C], F32, tag="bigps")
                for kc in range(CT):
                    for (c0, cn) in _chunks(C):
                        nc.tensor.matmul(
                            ps[:tn, c0:c0 + cn],
                            lhsT=xt16_t[kc][:, t0:t0 + tn],
                            rhs=wv_t[kc][:, c0:c0 + cn],
                            start=(kc == 0), stop=(kc == CT - 1))
                vt = vpool.tile([P, C], F16, tag=f"v16_{mt}")
                nc.scalar.copy(vt[:tn, :], ps[:tn, :])
                v16.append(vt)
            # prefetch next batch's x as soon as this batch's is consumed
            if b + 1 < BL:
                next_x = load_x(b + 1)

            # ── attention score diagonal: sd[h, n] = q_h·k_h per token
            sd_ps = psav.tile([P, 1024], F32, tag="avps")
            for kc in range(CT):
                qkm = qkpool.tile([P, NPAD], F32, tag="bigscratch")
                nc.vector.tensor_mul(qkm[:, :N], qT[kc][:], kT[kc][:])
                for (c0, cn) in _chunks(N):
                    nc.tensor.matmul(
                        sd_ps[:H, c0:c0 + cn],
                        lhsT=hsel_t[kc][:],
                        rhs=qkm[:, c0:c0 + cn],
                        start=(kc == 0), stop=(kc == CT - 1))
            sd_sb = spool.tile([H, N], F32, tag="sd_sb")
            nc.scalar.copy(sd_sb[:], sd_ps[:H, :N])

            # ── head pairs ───────────────────────────────────────────
            rowsum_all = spool.tile([H, N], F32, tag="rowsum_all")
            o16 = [opool.tile([P, N], F16, tag=f"o16_{i}", name=f"o16_{i}")
                   for i in range(CT)]
            pending = None   # (hp, accs, av_ps) awaiting rowsum+normalize

            def finish_pair(ctx_pair, all_pe=False):
                """Row sums + normalize for a pair — emitted one pair late so
                the PE-side reduction never waits on the DVE add chain."""
                hp_, accs_, av_ps_ = ctx_pair
                for par in (0, 1):
                    h = 2 * hp_ + par
                    r0 = par * HD
                    if par == 0 and not all_pe:
                        rs_red = ppool.tile([P, N], F32, tag="rs_red",
                                            name="rs_red", bufs=1)
                        nc.gpsimd.partition_all_reduce(
                            rs_red[:], accs_[par][:], channels=P,
                            reduce_op=bass_isa.ReduceOp.add)
                        rs_row = rs_red[0:1, :]
                    else:
                        rs_ps = pspool.tile([1, C], F32, tag="bigps",
                                            name="rsps")
                        for (c0, cn) in _chunks(N):
                            nc.tensor.matmul(
                                rs_ps[:1, c0:c0 + cn], lhsT=ones128[:],
                                rhs=accs_[par][:, c0:c0 + cn],
                                start=True, stop=True)
                        rs_sb = spool.tile([1, N], F32, tag=f"rs_sb{par}",
                                           name=f"rs_sb{par}")
                        nc.scalar.copy(rs_sb[:], rs_ps[:1, :N])
                        rs_row = rs_sb[:]
                    nc.scalar.dma_start(rowsum_all[h:h + 1, :], rs_row)
                    rec = spool.tile([1, N], F32, tag=f"rec{par}",
                                     name=f"rec{par}")
                    nc.vector.reciprocal_approx_fast(rec[:], rs_row)
                    bc = spool.tile([HD, N], F32, tag=f"bc{par}",
                                    name=f"bc{par}")
                    nc.gpsimd.partition_broadcast(bc[:], rec[:])
                    nc.vector.tensor_tensor(
                        o16[hp_][r0:r0 + HD, :], av_ps_[r0:r0 + HD, :N],
                        bc[:], OP.mult)

            for hp in range(H // 2):
                p16 = {}
                accs = {}
                # scores + exp for both heads, row-group packed
                for mt, (t0, tn) in enumerate(TOK_TILES):
                    s_ps = {}
                    for par in (0, 1):
                        r0 = par * HD
                        k_h = kT[hp][r0:r0 + HD, :]
                        q_h = qT[hp][r0:r0 + HD, :]
                        ps = pspool.tile([P, C], F32, tag="bigps",
                                         name=f"sps{par}")
                        for (c0, cn) in _chunks(N):
                            nc.tensor.matmul(
                                ps[:tn, c0:c0 + cn],
                                lhsT=k_h[:, t0:t0 + tn],
                                rhs=q_h[:, c0:c0 + cn],
                                start=True, stop=True,
                                tile_position=(r0, 0))
                        s_ps[par] = ps
                    for par in (0, 1):
                        p32 = ppool.tile([P, N], F32, tag="p32")
                        nc.scalar.activation(p32[:tn, :], s_ps[par][:tn, :N],
                                             ACTF.Exp, scale=SCALE)
                        pt = ppool.tile([P, N], F16, tag=f"p16_{par}_{mt}",
                                        name=f"p16_{par}_{mt}", bufs=1)
                        # split casts between DVE and ACT to balance engines
                        if mt % 2 == par:
                            nc.vector.tensor_copy(pt[:tn, :], p32[:tn, :])
                        else:
                            nc.scalar.copy(pt[:tn, :], p32[:tn, :])
                        p16[(par, mt)] = pt
                        if mt == 0:
                            a0 = ppool.tile([P, N], F32, tag=f"acc{par}",
                                            name=f"acc{par}", bufs=2)
                            nc.vector.tensor_copy(a0[:], p32[:])
                            accs[par] = a0
                        else:
                            nc.vector.tensor_add(accs[par][:tn, :],
                                                 accs[par][:tn, :],
                                                 p32[:tn, :])
                # rowsum+normalize of the PREVIOUS pair (its adds are done)
                if pending is not None:
                    finish_pair(pending)
                    pending = None
                # attn @ V for both heads, column-group packed
                av_ps = psav.tile([P, 1024], F32, tag="avps")
                for mt, (t0, tn) in enumerate(TOK_TILES):
                    for par in (0, 1):
                        h = 2 * hp + par
                        r0 = par * HD
                        for (c0, cn) in _chunks(N):
                            nc.tensor.matmul(
                                av_ps[r0:r0 + HD, c0:c0 + cn],
                                lhsT=v16[mt][:tn, h * HD:(h + 1) * HD],
                                rhs=p16[(par, mt)][:tn, c0:c0 + cn],
                                start=(mt == 0),
                                stop=(mt == len(TOK_TILES) - 1),
                                tile_position=(0, r0),
                                skip_group_check=True)
                pending = (hp, accs, av_ps)

            # last pair: both row sums on the PE (adds long done — no stall)
            finish_pair(pending, all_pe=True)
            pending = None

            # ── output projection + residual (PE keeps running) ──────
            y1s = []
            for mt, (t0, tn) in enumerate(TOK_TILES):
                y_ps = pspool.tile([P, C], F32, tag="bigps")
                for kc in range(CT):
                    for (c0, cn) in _chunks(C):
                        nc.tensor.matmul(
                            y_ps[:tn, c0:c0 + cn],
                            lhsT=o16[kc][:, t0:t0 + tn],
                            rhs=wp_t[kc][:, c0:c0 + cn],
                            start=(kc == 0), stop=(kc == CT - 1))
                xr_t = ypool.tile([P, C], F32, tag="xr_t", bufs=1)
                nc.sync.dma_start(xr_t[:tn, :], xr_e.ap()[b, t0:t0 + tn, :])
                y1 = ypool.tile([P, C], F32, tag=f"y1_{mt}", name=f"y1_{mt}",
                                bufs=1)
                nc.vector.tensor_add(y1[:tn, :], y_ps[:tn, :], bias_t[:tn, :])
                nc.vector.tensor_add(y1[:tn, :], y1[:tn, :], xr_t[:tn, :])
                y1s.append(y1)

            # ── ranking chain (DVE/GpSimd/DMA only — no PE stalls) ───
            pd = spool.tile([H, N], F32, tag="pd")
            nc.scalar.activation(pd[:], sd_sb[:], ACTF.Exp, scale=SCALE)
            rrec = spool.tile([H, N], F32, tag="rrec")
            nc.vector.reciprocal(rrec[:], rowsum_all[:])
            nc.vector.tensor_mul(pd[:], pd[:], rrec[:])
            a_red = spool.tile([H, N], F32, tag="a_red")
            nc.gpsimd.partition_all_reduce(
                a_red[:], pd[:], channels=H, reduce_op=bass_isa.ReduceOp.add)
            a_row = spool.tile([1, NPAD], F32, tag="a_row")
            nc.vector.tensor_copy(a_row[:, :N], a_red[0:1, :])
            nc.vector.memset(a_row[:, N:], NEG)
            nc.vector.memset(a_row[:, 0:1], 1.0e30)   # CLS always kept

            abc = spool.tile([P, NPAD], F32, tag="abc")
            nc.gpsimd.partition_broadcast(abc[:], a_row[:])
            a_dram = dpool.tile([1, NPAD], F32, tag="a_dram")
            nc.scalar.dma_start(a_dram[:], a_row[:])
            acp = spool.tile([P, 5], F32, tag="acp")
            nc.scalar.dma_start(
                acp[:], a_dram[:, :].rearrange("a (t p) -> (a p) t", p=P))
            rcnt = spool.tile([P, 5], F32, tag="rcnt")
            scratch = qkpool.tile([P, NPAD], F32, tag="bigscratch")
            keep = spool.tile([P, 5], F32, tag="keep")
            for t in range(5):
                nc.vector.tensor_scalar(
                    scratch[:], abc[:], acp[:, t:t + 1], None, OP.is_gt,
                    op1=OP.add, accum_out=rcnt[:, t:t + 1])
                nc.vector.tensor_single_scalar(
                    keep[:, t:t + 1], rcnt[:, t:t + 1], float(num_kept),
                    OP.is_lt)
            keep_dram = dpool.tile([1, NPAD], F32, tag="keep_dram")
            nc.scalar.dma_start(
                keep_dram[:, :].rearrange("a (t p) -> (a p) t", p=P), keep[:])
            keep_row = spool.tile([1, NPAD], F32, tag="keep_row")
            nc.scalar.dma_start(keep_row[:], keep_dram[:])
            pos_row = spool.tile([1, NPAD], F32, tag="pos_row")
            nc.vector.tensor_tensor_scan(
                pos_row[:], keep_row[:], zrow[:], 0.0, OP.add, OP.add)
            # scatter index: kept -> b*num_kept + pos - 1, dropped -> BIG
            nc.vector.tensor_single_scalar(
                pos_row[:], pos_row[:], float(b * num_kept - 1), OP.add)
            nc.vector.tensor_scalar(
                keep_row[:], keep_row[:], -BIG, BIG, OP.mult, op1=OP.add)
            nc.vector.tensor_tensor(pos_row[:], pos_row[:], keep_row[:],
                                    OP.add)
            idx_dram = dpool.tile([1, NPAD], F32, tag="idx_dram")
            nc.scalar.dma_start(idx_dram[:], pos_row[:])
            icp = spool.tile([P, 5], F32, tag="icp")
            nc.scalar.dma_start(
                icp[:], idx_dram[:, :].rearrange("a (t p) -> (a p) t", p=P))
            icpu = spool.tile([P, 5], U32, tag="icpu")
            nc.vector.tensor_copy(icpu[:], icp[:])

            for mt, (t0, tn) in enumerate(TOK_TILES):
                nc.gpsimd.indirect_dma_start(
                    out=out_flat,
                    out_offset=bass.IndirectOffsetOnAxis(
                        ap=icpu[:tn, mt:mt + 1], axis=0),
                    in_=y1s[mt][:tn, :],
                    in_offset=None,
                    bounds_check=BL * num_kept - 1,
                    oob_is_err=False)

    n_removed = _dedupe_ldweights(nc)
    nc.compile()
    return nc


def prep_inputs(x, qkv_w, proj_w, proj_b):
    """Host-side sharding + layout prep. Returns per-core in_maps."""
    x = np.ascontiguousarray(x, dtype=np.float32)
    qkv_w = np.asarray(qkv_w, dtype=np.float32)
    proj_w = np.asarray(proj_w, dtype=np.float32)
    proj_b = np.asarray(proj_b, dtype=np.float32)

    wq = np.ascontiguousarray(qkv_w[0:C].T)           # (c, outdim)
    wk = np.ascontiguousarray(qkv_w[C:2 * C].T)
    wv16 = np.ascontiguousarray(qkv_w[2 * C:3 * C].T).astype(np.float16)
    wp16 = np.ascontiguousarray(proj_w.T).astype(np.float16)
    biasb = np.ascontiguousarray(np.tile(proj_b[None, :], (P, 1)))
    hsel = np.zeros((C, H), dtype=np.float32)
    for h in range(H):
        hsel[h * HD:(h + 1) * HD, h] = 1.0

    in_maps = []
    for core in range(NCORES):
        xl = x[core * BL:(core + 1) * BL]             # (BL, N, C)
        xt = np.ascontiguousarray(xl.transpose(0, 2, 1))
        in_maps.append({
            "xt": xt,
            "xt16": xt.astype(np.float16),
            "xr": np.ascontiguousarray(xl),
            "wq": wq, "wk": wk, "wv16": wv16, "wp16": wp16,
            "biasb": biasb, "hsel": hsel,
        })
    return in_maps


_BUILD_CACHE = {}


def run(x, qkv_w, proj_w, proj_b, reduction_num, trace=False, **trace_kw):
    num_kept = N - int(reduction_num)
    if num_kept not in _BUILD_CACHE:
        _BUILD_CACHE[num_kept] = build(num_kept)
    nc = _BUILD_CACHE[num_kept]
    in_maps = prep_inputs(x, qkv_w, proj_w, proj_b)
    res = run_bass_kernel_spmd(nc, in_maps, core_ids=list(range(NCORES)),
                               trace=trace, **trace_kw)
    out = np.concatenate([res.results[c]["out"] for c in range(NCORES)],
                         axis=0)
    return out.astype(np.float32), res


def kernel(x, qkv_w, proj_w, proj_b, reduction_num):
    out, _ = run(x, qkv_w, proj_w, proj_b, reduction_num, trace=False)
    return out


# revision 28
# speedup vs baseline: 1.1204x; 1.1204x over previous
"""Trainium2 Bass kernel for nn_Attention_27943057228498 (sparse token-pruning
attention, ViT-style EViT).

Strategy: pure data parallelism over batch — 32 batches over 8 NeuronCores,
4 per core, no collectives.

Numerics: the top-k token selection compares attention diagonal values whose
boundary gaps are as small as ~6e-6 relative, so everything feeding the
ranking (QK projection, scores, softmax row sums, diagonal) runs in exact
fp32 on the PE (4 cycles/row). The output path (V projection, attn@V,
output projection) only needs ~1e-3, so it runs in fp16 (1 cycle/row).

PE packing: heads are processed in pairs. The pair's K^T/Q^T slices live at
partitions 0:64 / 64:128 of the same tile, so their K=64 score matmuls run
concurrently in disjoint PE row groups; the pair's attn@V matmuls (M=64)
run concurrently in disjoint column groups into one PSUM tile.

Token selection without sorting: per batch, token score a_j gets rank
R_j = #{i: a_i > a_j} via compare + row-reduce; keep = R < num_kept (CLS is
forced kept by pinning a_0 = +1e30); output positions are an inclusive prefix
scan of the keep mask; rows are emitted with an indirect-DMA scatter whose
out-of-bounds indices (dropped tokens) are silently discarded. The whole
selection chain runs on DVE/GpSimd/DMA so the PE never waits on it.
"""

import numpy as np

import concourse.bass as bass
import concourse.bass_isa as bass_isa
import concourse.tile as tile
import concourse.mybir as mybir
from concourse import bacc
from concourse.bass_utils import run_bass_kernel_spmd

# ── problem constants ────────────────────────────────────────────────
B, N, C = 32, 577, 768
H = 12
HD = C // H              # 64
NCORES = 8
BL = B // NCORES         # 4 batches per core
SCALE = HD ** -0.5       # 0.125 (exact power of two)

P = 128
TOK_TILES = [(0, 128), (128, 128), (256, 128), (384, 128), (512, 65)]  # 577
CT = C // P              # 6 channel tiles
NPAD = 640               # 577 padded to 5*128 for the rank machinery
BIG = 1.0e9              # scatter index for dropped rows (exact in fp32)
NEG = -1.0e30            # pad value below any real score

F32 = mybir.dt.float32
F16 = mybir.dt.float16
U32 = mybir.dt.uint32
OP = mybir.AluOpType
ACTF = mybir.ActivationFunctionType


def _dedupe_ldweights(nc):
    """Remove back-to-back duplicate PE Ldweights (same weights AP + array
    tile) so repeated matmuls on one stationary operand pay one load.

    Only deletes an Ldweights when (a) it has no semaphore waits/updates of
    its own, and (b) the PE weight state for its array-tile region is
    provably identical (no intervening Ldweights / self-loading Matmult
    overlapping that region).
    """

    def region(inst):
        tp = inst.tile_position or (0, 0)
        ts = inst.tile_size or (128, 128)
        return (tp[0], tp[0] + ts[0], tp[1], tp[1] + ts[1])

    def overlaps(r1, r2):
        return r1[0] < r2[1] and r2[0] < r1[1] and r1[2] < r2[3] and r2[2] < r1[3]

    removed = 0
    for blk in nc.m.functions[0].blocks:
        state = []   # list of (region, signature)
        keep_list = []
        for inst in blk.instructions:
            if not isinstance(inst, (mybir.InstLdweights, mybir.InstMatmult)):
                keep_list.append(inst)
                continue
            if isinstance(inst, mybir.InstLdweights):
                sig = (str(inst.ins[0]), inst.tile_position, inst.tile_size,
                       inst.is_transpose)
                r = region(inst)
                if (not inst.has_wait() and not inst.has_update()
                        and any(overlaps(r, r2) and s2 == sig and r2 == r
                                for r2, s2 in state)):
                    removed += 1
                    continue     # drop duplicate load
                state = [(r2, s2) for r2, s2 in state if not overlaps(r, r2)]
                state.append((r, sig))
                keep_list.append(inst)
            else:
                # self-loading matmul (fp32) clobbers its region
                if getattr(inst, "ldweights", None) is not False:
                    r = region(inst)
                    state = [(r2, s2) for r2, s2 in state
                             if not overlaps(r, r2)]
                keep_list.append(inst)
        if removed:
            blk.instructions[:] = keep_list
    return removed


def _chunks(total, limit=512):
    out = []
    c0 = 0
    while c0 < total:
        cn = min(limit, total - c0)
        out.append((c0, cn))
        c0 += cn
    return out


def build(num_kept: int):
    nc = bacc.Bacc("TRN2", target_bir_lowering=False, debug=False,
                   num_devices=NCORES)

    xt_e = nc.dram_tensor("xt", [BL, C, N], F32, kind="ExternalInput")
    xt16_e = nc.dram_tensor("xt16", [BL, C, N], F16, kind="ExternalInput")
    xr_e = nc.dram_tensor("xr", [BL, N, C], F32, kind="ExternalInput")
    wq_e = nc.dram_tensor("wq", [C, C], F32, kind="ExternalInput")
    wk_e = nc.dram_tensor("wk", [C, C], F32, kind="ExternalInput")
    wv16_e = nc.dram_tensor("wv16", [C, C], F16, kind="ExternalInput")
    wp16_e = nc.dram_tensor("wp16", [C, C], F16, kind="ExternalInput")
    bias_e = nc.dram_tensor("biasb", [P, C], F32, kind="ExternalInput")
    hsel_e = nc.dram_tensor("hsel", [C, H], F32, kind="ExternalInput")
    out_e = nc.dram_tensor("out", [BL, num_kept, C], F32, kind="ExternalOutput")
    out_flat = out_e.ap().rearrange("b n c -> (b n) c")

    from contextlib import ExitStack
    with tile.TileContext(nc) as tc, ExitStack() as ctx:
        wpool = ctx.enter_context(tc.tile_pool(name="weights", bufs=1))
        xpool = ctx.enter_context(tc.tile_pool(name="x", bufs=1))
        qkpool = ctx.enter_context(tc.tile_pool(name="qk", bufs=1))
        vpool = ctx.enter_context(tc.tile_pool(name="v", bufs=1))
        opool = ctx.enter_context(tc.tile_pool(name="o", bufs=1))
        ppool = ctx.enter_context(tc.tile_pool(name="p", bufs=2))
        spool = ctx.enter_context(tc.tile_pool(name="small", bufs=1))
        ypool = ctx.enter_context(tc.tile_pool(name="y", bufs=2))
        dpool = ctx.enter_context(tc.tile_pool(name="dram", bufs=2,
                                               space="DRAM"))
        pspool = ctx.enter_context(tc.tile_pool(name="ps", bufs=3,
                                                space="PSUM"))
        psav = ctx.enter_context(tc.tile_pool(name="psav", bufs=1,
                                              space="PSUM"))

        # ── resident weights / constants ─────────────────────────────
        wq_t, wk_t, wv_t, wp_t, hsel_t = [], [], [], [], []
        for i in range(CT):
            w1 = wpool.tile([P, C], F32, tag=f"wq{i}")
            nc.sync.dma_start(w1[:], wq_e.ap()[i * P:(i + 1) * P, :])
            wq_t.append(w1)
            w2 = wpool.tile([P, C], F32, tag=f"wk{i}")
            nc.sync.dma_start(w2[:], wk_e.ap()[i * P:(i + 1) * P, :])
            wk_t.append(w2)
            w3 = wpool.tile([P, C], F16, tag=f"wv{i}")
            nc.sync.dma_start(w3[:], wv16_e.ap()[i * P:(i + 1) * P, :])
            wv_t.append(w3)
            w4 = wpool.tile([P, C], F16, tag=f"wp{i}")
            nc.sync.dma_start(w4[:], wp16_e.ap()[i * P:(i + 1) * P, :])
            wp_t.append(w4)
            w5 = wpool.tile([P, H], F32, tag=f"hs{i}")
            nc.sync.dma_start(w5[:], hsel_e.ap()[i * P:(i + 1) * P, :])
            hsel_t.append(w5)
        bias_t = wpool.tile([P, C], F32, tag="bias")
        nc.sync.dma_start(bias_t[:], bias_e.ap())
        zrow = wpool.tile([1, NPAD], F32, tag="zrow")
        nc.vector.memset(zrow[:], 0.0)
        ones128 = wpool.tile([P, 1], F32, tag="ones128")
        nc.vector.memset(ones128[:], 1.0)

        def load_x(b):
            xt_t, xt16_t = [], []
            for i in range(CT):
                t1 = xpool.tile([P, N], F32, tag=f"xt{i}", name=f"xt{i}")
                nc.sync.dma_start(t1[:], xt_e.ap()[b, i * P:(i + 1) * P, :])
                xt_t.append(t1)
                t2 = xpool.tile([P, N], F16, tag=f"xt16{i}", name=f"xt16{i}")
                nc.sync.dma_start(t2[:], xt16_e.ap()[b, i * P:(i + 1) * P, :])
                xt16_t.append(t2)
            return xt_t, xt16_t

        next_x = load_x(0)
        for b in range(BL):
            xt_t, xt16_t = next_x

            # ── Q/K projections (fp32): qT/kT[mo] = (C-tile, N) ──────
            qT, kT = [], []
            for w_t, dst in ((wq_t, qT), (wk_t, kT)):
                for mo in range(CT):
                    ps = pspool.tile([P, C], F32, tag="bigps")
                    for kc in range(CT):
                        for (c0, cn) in _chunks(N):
                            nc.tensor.matmul(
                                ps[:, c0:c0 + cn],
                                lhsT=w_t[kc][:, mo * P:(mo + 1) * P],
                                rhs=xt_t[kc][:, c0:c0 + cn],
                                start=(kc == 0), stop=(kc == CT - 1))
                    sb = qkpool.tile([P, N], F32,
                                     tag=f"{'q' if dst is qT else 'k'}T{mo}")
                    nc.scalar.copy(sb[:], ps[:, :N])
                    dst.append(sb)

            # ── V projection (fp16) → v16[mt] = (m, C) ───────────────
            v16 = []
            for mt, (t0, tn) in enumerate(TOK_TILES):
                ps = pspool.tile([P, C], F32, tag="bigps")
                for kc in range(CT):
                    for (c0, cn) in _chunks(C):
                        nc.tensor.matmul(
                            ps[:tn, c0:c0 + cn],
                            lhsT=xt16_t[kc][:, t0:t0 + tn],
                            rhs=wv_t[kc][:, c0:c0 + cn],
                            start=(kc == 0), stop=(kc == CT - 1))
                vt = vpool.tile([P, C], F16, tag=f"v16_{mt}")
                nc.scalar.copy(vt[:tn, :], ps[:tn, :])
                v16.append(vt)
            # prefetch next batch's x as soon as this batch's is consumed
            if b + 1 < BL:
                next_x = load_x(b + 1)

            # ── attention score diagonal: sd[h, n] = q_h·k_h per token
            sd_ps = psav.tile([P, 1024], F32, tag="avps")
            for kc in range(CT):
                qkm = qkpool.tile([P, NPAD], F32, tag="bigscratch")
                nc.vector.tensor_mul(qkm[:, :N], qT[kc][:], kT[kc][:])
                for (c0, cn) in _chunks(N):
                    nc.tensor.matmul(
                        sd_ps[:H, c0:c0 + cn],
                        lhsT=hsel_t[kc][:],
                        rhs=qkm[:, c0:c0 + cn],
                        start=(kc == 0), stop=(kc == CT - 1))
            sd_sb = spool.tile([H, N], F32, tag="sd_sb")
            nc.scalar.copy(sd_sb[:], sd_ps[:H, :N])

            # ── head pairs ───────────────────────────────────────────
            rowsum_all = spool.tile([H, N], F32, tag="rowsum_all")
            o16 = [opool.tile([P, N], F16, tag=f"o16_{i}", name=f"o16_{i}")
                   for i in range(CT)]
            pending = None   # (hp, accs, av_ps) awaiting rowsum+normalize

            def finish_pair(ctx_pair, all_pe=False):
                """Row sums + normalize for a pair — emitted one pair late so
                the PE-side reduction never waits on the DVE add chain."""
                hp_, accs_, av_ps_ = ctx_pair
                for par in (0, 1):
                    h = 2 * hp_ + par
                    r0 = par * HD
                    if par == 0 and not all_pe:
                        rs_red = ppool.tile([P, N], F32, tag="rs_red",
                                            name="rs_red", bufs=1)
                        nc.gpsimd.partition_all_reduce(
                            rs_red[:], accs_[par][:], channels=P,
                            reduce_op=bass_isa.ReduceOp.add)
                        rs_row = rs_red[0:1, :]
                    else:
                        rs_ps = pspool.tile([1, C], F32, tag="bigps",
                                            name="rsps")
                        for (c0, cn) in _chunks(N):
                            nc.tensor.matmul(
                                rs_ps[:1, c0:c0 + cn], lhsT=ones128[:],
                                rhs=accs_[par][:, c0:c0 + cn],
                                start=True, stop=True)
                        rs_sb = spool.tile([1, N], F32, tag=f"rs_sb{par}",
                                           name=f"rs_sb{par}")
                        nc.scalar.copy(rs_sb[:], rs_ps[:1, :N])
                        rs_row = rs_sb[:]
                    nc.scalar.dma_start(rowsum_all[h:h + 1, :], rs_row)
                    rec = spool.tile([1, N], F32, tag=f"rec{par}",
                                     name=f"rec{par}")
                    nc.vector.reciprocal_approx_fast(rec[:], rs_row)
                    bc = spool.tile([HD, N], F32, tag=f"bc{par}",
                                    name=f"bc{par}")
                    nc.gpsimd.partition_broadcast(bc[:], rec[:])
                    nc.vector.tensor_tensor(
                        o16[hp_][r0:r0 + HD, :], av_ps_[r0:r0 + HD, :N],
                        bc[:], OP.mult)

            for hp in range(H // 2):
                p16 = {}
                accs = {}
                # scores + exp for both heads, row-group packed
                for mt, (t0, tn) in enumerate(TOK_TILES):
                    s_ps = {}
                    for par in (0, 1):
                        r0 = par * HD
                        k_h = kT[hp][r0:r0 + HD, :]
                        q_h = qT[hp][r0:r0 + HD, :]
                        ps = pspool.tile([P, C], F32, tag="bigps",
                                         name=f"sps{par}")
                        for (c0, cn) in _chunks(N):
                            nc.tensor.matmul(
                                ps[:tn, c0:c0 + cn],
                                lhsT=k_h[:, t0:t0 + tn],
                                rhs=q_h[:, c0:c0 + cn],
                                start=True, stop=True,
                                tile_position=(r0, 0))
                        s_ps[par] = ps
                    for par in (0, 1):
                        p32 = ppool.tile([P, N], F32, tag="p32")
                        nc.scalar.activation(p32[:tn, :], s_ps[par][:tn, :N],
                                             ACTF.Exp, scale=SCALE)
                        pt = ppool.tile([P, N], F16, tag=f"p16_{par}_{mt}",
                                        name=f"p16_{par}_{mt}", bufs=1)
                        # split casts between DVE and ACT to balance engines
                        if mt % 2 == par:
                            nc.vector.tensor_copy(pt[:tn, :], p32[:tn, :])
                        else:
                            nc.scalar.copy(pt[:tn, :], p32[:tn, :])
                        p16[(par, mt)] = pt
                        if mt == 0:
                            a0 = ppool.tile([P, N], F32, tag=f"acc{par}",
                                            name=f"acc{par}", bufs=2)
                            nc.vector.tensor_copy(a0[:], p32[:])
                            accs[par] = a0
                        else:
                            nc.vector.tensor_add(accs[par][:tn, :],
                                                 accs[par][:tn, :],
                                                 p32[:tn, :])
                # rowsum+normalize of the PREVIOUS pair (its adds are done)
                if pending is not None:
                    finish_pair(pending, all_pe=True)
                    pending = None
                # attn @ V for both heads, column-group packed
                av_ps = psav.tile([P, 1024], F32, tag="avps")
                for mt, (t0, tn) in enumerate(TOK_TILES):
                    for par in (0, 1):
                        h = 2 * hp + par
                        r0 = par * HD
                        for (c0, cn) in _chunks(N):
                            nc.tensor.matmul(
                                av_ps[r0:r0 + HD, c0:c0 + cn],
                                lhsT=v16[mt][:tn, h * HD:(h + 1) * HD],
                                rhs=p16[(par, mt)][:tn, c0:c0 + cn],
                                start=(mt == 0),
                                stop=(mt == len(TOK_TILES) - 1),
                                tile_position=(0, r0),
                                skip_group_check=True)
                pending = (hp, accs, av_ps)

            # last pair: both row sums on the PE (adds long done — no stall)
            finish_pair(pending, all_pe=True)
            pending = None

            # ── output projection + residual (PE keeps running) ──────
            y1s = []
            for mt, (t0, tn) in enumerate(TOK_TILES):
                y_ps = pspool.tile([P, C], F32, tag="bigps")
                for kc in range(CT):
                    for (c0, cn) in _chunks(C):
                        nc.tensor.matmul(
                            y_ps[:tn, c0:c0 + cn],
                            lhsT=o16[kc][:, t0:t0 + tn],
                            rhs=wp_t[kc][:, c0:c0 + cn],
                            start=(kc == 0), stop=(kc == CT - 1))
                xr_t = ypool.tile([P, C], F32, tag="xr_t", bufs=1)
                nc.sync.dma_start(xr_t[:tn, :], xr_e.ap()[b, t0:t0 + tn, :])
                y1 = ypool.tile([P, C], F32, tag=f"y1_{mt}", name=f"y1_{mt}",
                                bufs=1)
                nc.vector.tensor_add(y1[:tn, :], y_ps[:tn, :], bias_t[:tn, :])
                nc.vector.tensor_add(y1[:tn, :], y1[:tn, :], xr_t[:tn, :])
                y1s.append(y1)

            # ── ranking chain (DVE/GpSimd/DMA only — no PE stalls) ───
            pd = spool.tile([H, N], F32, tag="pd")
            nc.scalar.activation(pd[:], sd_sb[:], ACTF.Exp, scale=SCALE)
            rrec = spool.tile([H, N], F32, tag="rrec")
            nc.vector.reciprocal(rrec[:], rowsum_all[:])
            nc.vector.tensor_mul(pd[:], pd[:], rrec[:])
            a_red = spool.tile([H, N], F32, tag="a_red")
            nc.gpsimd.partition_all_reduce(
                a_red[:], pd[:], channels=H, reduce_op=bass_isa.ReduceOp.add)
            a_row = spool.tile([1, NPAD], F32, tag="a_row")
            nc.vector.tensor_copy(a_row[:, :N], a_red[0:1, :])
            nc.vector.memset(a_row[:, N:], NEG)
            nc.vector.memset(a_row[:, 0:1], 1.0e30)   # CLS always kept

            abc = spool.tile([P, NPAD], F32, tag="abc")
            nc.gpsimd.partition_broadcast(abc[:], a_row[:])
            a_dram = dpool.tile([1, NPAD], F32, tag="a_dram")
            nc.scalar.dma_start(a_dram[:], a_row[:])
            acp = spool.tile([P, 5], F32, tag="acp")
            nc.scalar.dma_start(
                acp[:], a_dram[:, :].rearrange("a (t p) -> (a p) t", p=P))
            rcnt = spool.tile([P, 5], F32, tag="rcnt")
            scratch = qkpool.tile([P, NPAD], F32, tag="bigscratch")
            keep = spool.tile([P, 5], F32, tag="keep")
            for t in range(5):
                nc.vector.tensor_scalar(
                    scratch[:], abc[:], acp[:, t:t + 1], None, OP.is_gt,
                    op1=OP.add, accum_out=rcnt[:, t:t + 1])
                nc.vector.tensor_single_scalar(
                    keep[:, t:t + 1], rcnt[:, t:t + 1], float(num_kept),
                    OP.is_lt)
            keep_dram = dpool.tile([1, NPAD], F32, tag="keep_dram")
            nc.scalar.dma_start(
                keep_dram[:, :].rearrange("a (t p) -> (a p) t", p=P), keep[:])
            keep_row = spool.tile([1, NPAD], F32, tag="keep_row")
            nc.scalar.dma_start(keep_row[:], keep_dram[:])
            pos_row = spool.tile([1, NPAD], F32, tag="pos_row")
            nc.vector.tensor_tensor_scan(
                pos_row[:], keep_row[:], zrow[:], 0.0, OP.add, OP.add)
            # scatter index: kept -> b*num_kept + pos - 1, dropped -> BIG
            nc.vector.tensor_single_scalar(
                pos_row[:], pos_row[:], float(b * num_kept - 1), OP.add)
            nc.vector.tensor_scalar(
                keep_row[:], keep_row[:], -BIG, BIG, OP.mult, op1=OP.add)
            nc.vector.tensor_tensor(pos_row[:], pos_row[:], keep_row[:],
                                    OP.add)
            idx_dram = dpool.tile([1, NPAD], F32, tag="idx_dram")
            nc.scalar.dma_start(idx_dram[:], pos_row[:])
            icp = spool.tile([P, 5], F32, tag="icp")
            nc.scalar.dma_start(
                icp[:], idx_dram[:, :].rearrange("a (t p) -> (a p) t", p=P))
            icpu = spool.tile([P, 5], U32, tag="icpu")
            nc.vector.tensor_copy(icpu[:], icp[:])

            for mt, (t0, tn) in enumerate(TOK_TILES):
                nc.gpsimd.indirect_dma_start(
                    out=out_flat,
                    out_offset=bass.IndirectOffsetOnAxis(
                        ap=icpu[:tn, mt:mt + 1], axis=0),
                    in_=y1s[mt][:tn, :],
                    in_offset=None,
                    bounds_check=BL * num_kept - 1,
                    oob_is_err=False)

    n_removed = _dedupe_ldweights(nc)
    nc.compile()
    return nc


def prep_inputs(x, qkv_w, proj_w, proj_b):
    """Host-side sharding + layout prep. Returns per-core in_maps."""
    x = np.ascontiguousarray(x, dtype=np.float32)
    qkv_w = np.asarray(qkv_w, dtype=np.float32)
    proj_w = np.asarray(proj_w, dtype=np.float32)
    proj_b = np.asarray(proj_b, dtype=np.float32)

    wq = np.ascontiguousarray(qkv_w[0:C].T)           # (c, outdim)
    wk = np.ascontiguousarray(qkv_w[C:2 * C].T)
    wv16 = np.ascontiguousarray(qkv_w[2 * C:3 * C].T).astype(np.float16)
    wp16 = np.ascontiguousarray(proj_w.T).astype(np.float16)
    biasb = np.ascontiguousarray(np.tile(proj_b[None, :], (P, 1)))
    hsel = np.zeros((C, H), dtype=np.float32)
    for h in range(H):
        hsel[h * HD:(h + 1) * HD, h] = 1.0

    in_maps = []
    for core in range(NCORES):
        xl = x[core * BL:(core + 1) * BL]             # (BL, N, C)
        xt = np.ascontiguousarray(xl.transpose(0, 2, 1))
        in_maps.append({
            "xt": xt,
            "xt16": xt.astype(np.float16),
            "xr": np.ascontiguousarray(xl),
            "wq": wq, "wk": wk, "wv16": wv16, "wp16": wp16,
            "biasb": biasb, "hsel": hsel,
        })
    return in_maps


_BUILD_CACHE = {}


def run(x, qkv_w, proj_w, proj_b, reduction_num, trace=False, **trace_kw):
    num_kept = N - int(reduction_num)
    if num_kept not in _BUILD_CACHE:
        _BUILD_CACHE[num_kept] = build(num_kept)
    nc = _BUILD_CACHE[num_kept]
    in_maps = prep_inputs(x, qkv_w, proj_w, proj_b)
    res = run_bass_kernel_spmd(nc, in_maps, core_ids=list(range(NCORES)),
                               trace=trace, **trace_kw)
    out = np.concatenate([res.results[c]["out"] for c in range(NCORES)],
                         axis=0)
    return out.astype(np.float32), res


def kernel(x, qkv_w, proj_w, proj_b, reduction_num):
    out, _ = run(x, qkv_w, proj_w, proj_b, reduction_num, trace=False)
    return out
